# revision 1
# baseline (speedup 1.0000x reference)
"""Pre-LN transformer block (B=2,T=2048,C=1024,H=16) on 8 TRN2 NeuronCores.

Two SPMD launches:
  L1: tensor-parallel over heads (2 heads/core) - LN1 stats on-chip
      (replicated, folded algebraically into the QKV matmuls), causal
      attention with on-chip softmax, normalized attn^T output per core.
  L2: parallel over token rows (512 rows/core) - output projection +
      residual, LN2, FFN (relu) + residual.
Host work between launches is just resharding: slicing/concat and dtype
casts. All matmuls run in bf16 with fp32 PSUM accumulation; residuals
are carried in fp32.
"""
"""Transformer block on 8 TRN2 cores: L1 = head-parallel attention, L2 = row-parallel proj+FFN."""
import contextlib
import numpy as np
import ml_dtypes

import concourse.bass as bass
import concourse.mybir as mybir
import concourse.tile as tile
from concourse import bacc
from concourse.masks import make_identity

bf16 = ml_dtypes.bfloat16
FP32 = mybir.dt.float32
BF16 = mybir.dt.bfloat16
AF = mybir.ActivationFunctionType

B, T, C, H = 2, 2048, 1024, 16
HS = C // H          # 64
NCORES = 8
HPC = H // NCORES    # 2 heads per core
TOK = B * T          # 4096
EPS = 1e-5
CT = C // 128        # 8 c-tiles
NCH = TOK // 512     # 8 512-col chunks of token axis
QB = 512             # query block
ROWS = TOK // NCORES # 512 rows per core in L2
HID = 4 * C          # 4096
HT = HID // 128      # 32 hidden tiles
MT = ROWS // 128     # 4 token tiles in L2


def _emit_attnv(nc, vts, vcol, item):
    pasl, koff, pr, c0, is_start, is_stop = item
    nc.tensor.matmul(pasl[:, c0:] if c0 else pasl, vts[koff // 128][:, vcol],
                     pr[:, c0:] if c0 else pr,
                     start=is_start, stop=is_stop, skip_group_check=True)


def build_l1(debug=False, use_beta=True):
    nc = bacc.Bacc("TRN2", target_bir_lowering=False, debug=False, num_devices=NCORES)
    xt_d = nc.dram_tensor("xt", [C, TOK], BF16, kind="ExternalInput").ap()
    wq_d = nc.dram_tensor("wq", [C, 128], BF16, kind="ExternalInput").ap()
    wk_d = nc.dram_tensor("wk", [C, 128], BF16, kind="ExternalInput").ap()
    wv_d = nc.dram_tensor("wv", [C, 128], BF16, kind="ExternalInput").ap()
    # negated column sums of wq/wk/wv and W.T @ beta1, all [128,1] fp32
    nws_d = nc.dram_tensor("nws", [128, 3], FP32, kind="ExternalInput").ap()
    wb_d = nc.dram_tensor("wb", [128, 3], FP32, kind="ExternalInput").ap()
    tri_d = nc.dram_tensor("tri", [128, 128], BF16, kind="ExternalInput").ap()
    out_d = nc.dram_tensor("attn_out", [128, TOK], BF16, kind="ExternalOutput").ap()
    if debug:
        dbg_rstd = nc.dram_tensor("dbg_rstd", [128, TOK], FP32, kind="ExternalOutput").ap()
        dbg_mur = nc.dram_tensor("dbg_mur", [128, TOK], FP32, kind="ExternalOutput").ap()
        dbg_qt = nc.dram_tensor("dbg_qt", [128, TOK], BF16, kind="ExternalOutput").ap()
        dbg_kt = nc.dram_tensor("dbg_kt", [128, TOK], BF16, kind="ExternalOutput").ap()
        dbg_v = nc.dram_tensor("dbg_v", [128, 130], BF16, kind="ExternalOutput").ap()

    with tile.TileContext(nc) as tc, contextlib.ExitStack() as ctx:
        consts = ctx.enter_context(tc.tile_pool(name="consts", bufs=1))
        hpool = ctx.enter_context(tc.tile_pool(name="hT", bufs=1))
        stats = ctx.enter_context(tc.tile_pool(name="stats", bufs=1))
        tmp = ctx.enter_context(tc.tile_pool(name="tmp", bufs=3))
        qkv = ctx.enter_context(tc.tile_pool(name="qkv", bufs=1))
        probs_p = ctx.enter_context(tc.tile_pool(name="probs", bufs=10))
        attn_sb_p = ctx.enter_context(tc.tile_pool(name="attn_sb", bufs=3))
        ps_rot = ctx.enter_context(tc.tile_pool(name="ps_rot", bufs=5, space="PSUM"))
        ps_acc = ctx.enter_context(tc.tile_pool(name="ps_acc", bufs=3, space="PSUM"))

        # ---- constants ----
        ones_sb = consts.tile([128, 128], BF16)
        nc.vector.memset(ones_sb, 1.0)
        eps_sb = consts.tile([128, 1], FP32)
        nc.vector.memset(eps_sb, EPS)
        ident = consts.tile([128, 128], BF16)
        make_identity(nc, ident)

        # PE warm-up spin: keep HAM busy while input DMAs stream
        warm_ps = ps_acc.tile([128, 512], FP32, tag="pa")
        for _ in range(20):
            nc.tensor.matmul(warm_ps[:, 0:128], ones_sb, ones_sb[:, 0:128], start=True, stop=True)

        wq_sb = consts.tile([128, CT, 128], BF16)
        nc.sync.dma_start(out=wq_sb, in_=wq_d.rearrange("(a p) m -> p a m", p=128))
        wk_sb = consts.tile([128, CT, 128], BF16)
        nc.sync.dma_start(out=wk_sb, in_=wk_d.rearrange("(a p) m -> p a m", p=128))
        wv_sb = consts.tile([128, CT, 128], BF16)
        nc.sync.dma_start(out=wv_sb, in_=wv_d.rearrange("(a p) m -> p a m", p=128))
        nws_sb = consts.tile([128, 3], FP32)
        nc.sync.dma_start(out=nws_sb, in_=nws_d)
        wb_sb = consts.tile([128, 3], FP32)
        nc.sync.dma_start(out=wb_sb, in_=wb_d)
        tri_sb = consts.tile([128, 128], BF16)
        nc.sync.dma_start(out=tri_sb, in_=tri_d)

        # ---- load xT (chunk-major so chunk j of every c-tile lands early) ----
        xts = []
        for ci in range(CT):
            t = hpool.tile([128, TOK], BF16, tag=f"hT{ci}")
            xts.append(t)
        for j in range(NCH):
            sl = slice(j * 512, (j + 1) * 512)
            for ci in range(CT):
                nc.sync.dma_start(out=xts[ci][:, sl], in_=xt_d[ci * 128:(ci + 1) * 128, sl])

        # ---- LN1 stats (transposed orientation; ones[128,128] stationary makes
        # every PSUM partition carry the same column sums -> stats born broadcast) ----
        rstd_b = stats.tile([128, TOK], BF16, tag="rstd_b")
        murstd_b = stats.tile([128, TOK], BF16, tag="murstd_b")
        for j in range(NCH):
            sl = slice(j * 512, (j + 1) * 512)
            ps_sum = ps_rot.tile([128, 512], FP32, tag="mm")
            ps_sq = ps_rot.tile([128, 512], FP32, tag="mm")
            for ci in range(CT):
                sq = tmp.tile([128, 512], BF16, tag="sq")
                nc.vector.tensor_mul(sq, xts[ci][:, sl], xts[ci][:, sl])
                nc.tensor.matmul(ps_sum, ones_sb, xts[ci][:, sl],
                                 start=(ci == 0), stop=(ci == CT - 1))
                nc.tensor.matmul(ps_sq, ones_sb, sq,
                                 start=(ci == 0), stop=(ci == CT - 1))
            mu = tmp.tile([128, 512], FP32, tag="mu")
            nc.scalar.mul(out=mu, in_=ps_sum, mul=1.0 / C)
            var = tmp.tile([128, 512], FP32, tag="var")
            nc.scalar.mul(out=var, in_=ps_sq, mul=1.0 / C)
            mu2 = tmp.tile([128, 512], FP32, tag="mu2")
            nc.vector.tensor_mul(mu2, mu, mu)
            nc.vector.tensor_sub(var, var, mu2)
            # rstd = (var+eps)^-0.5 = Exp(-0.5*Ln(var+eps)): stays on the
            # natural_log_exp_and_others ACT table (no Sqrt table reloads)
            nc.scalar.activation(out=var, in_=var, func=AF.Ln, bias=eps_sb, scale=1.0)
            nc.scalar.activation(out=rstd_b[:, sl], in_=var, func=AF.Exp, scale=-0.5)
            nc.vector.tensor_mul(murstd_b[:, sl], mu, rstd_b[:, sl])

        # ---- QT/KT/VT on RAW xT; LN folded in afterwards:
        #      QT = rstd*(Wq.T@xT) + (-colsum(Wq))*murstd + Wq.T@beta ----
        qt_sb = qkv.tile([128, TOK], BF16, tag="qt")
        kt_sb = qkv.tile([128, TOK], BF16, tag="kt")
        vt_sb = qkv.tile([128, TOK], BF16, tag="vt")
        for half in range(2):
            for wsb, idx, tsb in ((wq_sb, 0, qt_sb), (wk_sb, 1, kt_sb), (wv_sb, 2, vt_sb)):
                pss = []
                for jl in range(4):
                    p = ps_rot.tile([128, 512], FP32, tag="mm")
                    pss.append(p)
                # stationary wsb[:, ci, :] is loaded once per ci and streams
                # all four 512-token chunks (4x fewer LDWEIGHTS)
                for ci in range(CT):
                    for jl in range(4):
                        j = half * 4 + jl
                        sl = slice(j * 512, (j + 1) * 512)
                        nc.tensor.matmul(pss[jl], wsb[:, ci, :], xts[ci][:, sl],
                                         start=(ci == 0), stop=(ci == CT - 1))
                for jl in range(4):
                    j = half * 4 + jl
                    sl = slice(j * 512, (j + 1) * 512)
                    nc.scalar.copy(out=tsb[:, sl], in_=pss[jl])
                    nc.vector.tensor_mul(tsb[:, sl], tsb[:, sl], rstd_b[:, sl])
                    nc.vector.scalar_tensor_tensor(
                        tsb[:, sl], murstd_b[:, sl], nws_sb[:, idx:idx + 1], tsb[:, sl],
                        op0=mybir.AluOpType.mult, op1=mybir.AluOpType.add)
                    if use_beta:
                        nc.vector.tensor_scalar_add(tsb[:, sl], tsb[:, sl],
                                                    wb_sb[:, idx:idx + 1])

        # ---- V' tiles [128, 130] = [v_h0 | ones | v_h1 | ones] via PE transpose of VT ----
        vts = []
        for tt in range(TOK // 128):
            vt = qkv.tile([128, 130], BF16, tag=f"v{tt}")
            ptv = ps_rot.tile([128, 128], BF16, tag="mm")
            nc.tensor.transpose(ptv, vt_sb[:, tt * 128:(tt + 1) * 128], ident)
            nc.scalar.copy(out=vt.rearrange("p (g c) -> p g c", g=2)[:, :, 0:64],
                           in_=ptv.rearrange("p (g c) -> p g c", g=2))
            nc.vector.memset(vt[:, 64:65], 1.0)
            nc.vector.memset(vt[:, 129:130], 1.0)
            vts.append(vt)

        if debug:
            nc.sync.dma_start(out=dbg_rstd, in_=rstd_b)
            nc.sync.dma_start(out=dbg_mur, in_=murstd_b)
            nc.sync.dma_start(out=dbg_qt, in_=qt_sb)
            nc.sync.dma_start(out=dbg_kt, in_=kt_sb)
            nc.sync.dma_start(out=dbg_v, in_=vts[0])

        # ---- attention: two (batch,head) groups interleaved so PE always has
        # independent work while exp/mask complete ----
        scale = C ** -0.5

        def attn_group(b, hl):
            hsl = slice(hl * 64, (hl + 1) * 64)
            vcol = slice(hl * 65, hl * 65 + 65)
            pend = []

            def _flush(item):
                pa_, q0_, koff_, pr_, c0_, st_, sp_ = item
                nc.tensor.matmul(pa_[:, c0_:] if c0_ else pa_,
                                 vts[koff_ // 128][:, vcol],
                                 pr_[:, c0_:] if c0_ else pr_,
                                 start=st_, stop=sp_, skip_group_check=True)
                if sp_:
                    asb = attn_sb_p.tile([65, 512], FP32, tag="asb")
                    # ACT drains PSUM ~6x faster than DVE and Copy shares the
                    # Exp activation table (no reload)
                    nc.scalar.copy(out=asb, in_=pa_)
                    den = attn_sb_p.tile([1, 512], FP32, tag="den")
                    nc.gpsimd.dma_start(out=den, in_=asb[64:65, :])
                    rec = attn_sb_p.tile([64, 512], FP32, tag="rec")
                    nc.gpsimd.partition_broadcast(rec, den, channels=64)
                    recf = attn_sb_p.tile([64, 512], FP32, tag="recf")
                    nc.vector.reciprocal_approx_fast(recf, rec)
                    ao = attn_sb_p.tile([64, 512], BF16, tag="ao")
                    nc.vector.tensor_mul(ao, asb[0:64, :], recf)
                    nc.scalar.dma_start(out=out_d[hl * 64:(hl + 1) * 64, q0_:q0_ + QB],
                                        in_=ao)

            for j in range(T // QB):
                q0 = b * T + j * QB
                pa = ps_acc.tile([65, 512], FP32, tag="pa")
                nkt = 4 * (j + 1)
                for kt in range(nkt):
                    koff = b * T + kt * 128
                    d = kt - 4 * j
                    c0 = 128 * d if d > 0 else 0
                    ps = ps_rot.tile([128, 512], FP32, tag="mm")
                    nc.tensor.matmul(ps[:, c0:], kt_sb[hsl, koff:koff + 128],
                                     qt_sb[hsl, q0 + c0:q0 + QB],
                                     start=True, stop=True)
                    pr = probs_p.tile([128, 512], BF16, tag="pr")
                    nc.scalar.activation(out=pr[:, c0:], in_=ps[:, c0:],
                                         func=AF.Exp, scale=scale)
                    if d >= 0:
                        nc.vector.tensor_mul(pr[:, 128 * d:128 * (d + 1)],
                                             pr[:, 128 * d:128 * (d + 1)], tri_sb)
                    pend.append((pa, q0, koff, pr, c0, kt == 0, kt == nkt - 1))
                    if len(pend) > 4:
                        _flush(pend.pop(0))
                    yield
            while pend:
                _flush(pend.pop(0))
                yield

        for b in range(B):
            gens = [attn_group(b, 0), attn_group(b, 1)]
            while gens:
                for g in list(gens):
                    try:
                        next(g)
                    except StopIteration:
                        gens.remove(g)
    nc.compile()
    return nc


def build_l2():
    nc = bacc.Bacc("TRN2", target_bir_lowering=False, debug=False, num_devices=NCORES)
    at_d = nc.dram_tensor("at", [C, ROWS], BF16, kind="ExternalInput").ap()
    wp_d = nc.dram_tensor("wp", [C, C], BF16, kind="ExternalInput").ap()
    xr_d = nc.dram_tensor("xr", [ROWS, C], FP32, kind="ExternalInput").ap()
    w1_d = nc.dram_tensor("w1q", [128, HT, CT * 128], BF16, kind="ExternalInput").ap()
    w2_d = nc.dram_tensor("w2", [HID, C], BF16, kind="ExternalInput").ap()
    b1_d = nc.dram_tensor("b1", [HID, 1], FP32, kind="ExternalInput").ap()
    beta2_d = nc.dram_tensor("beta2", [1, C], FP32, kind="ExternalInput").ap()
    b2_d = nc.dram_tensor("b2", [1, C], FP32, kind="ExternalInput").ap()
    out_d = nc.dram_tensor("out_rows", [ROWS, C], FP32, kind="ExternalOutput").ap()

    with tile.TileContext(nc) as tc, contextlib.ExitStack() as ctx:
        consts = ctx.enter_context(tc.tile_pool(name="consts", bufs=1))
        persist = ctx.enter_context(tc.tile_pool(name="persist", bufs=1))
        wstream = ctx.enter_context(tc.tile_pool(name="wstream", bufs=3))
        tmp = ctx.enter_context(tc.tile_pool(name="tmp", bufs=3))
        small = ctx.enter_context(tc.tile_pool(name="small", bufs=4))
        ps_p = ctx.enter_context(tc.tile_pool(name="ps_p", bufs=3, space="PSUM"))
        ps_tr = ctx.enter_context(tc.tile_pool(name="ps_tr", bufs=1, space="PSUM"))
        ps_o = ctx.enter_context(tc.tile_pool(name="ps_o", bufs=1, space="PSUM"))

        ident = consts.tile([128, 128], BF16)
        make_identity(nc, ident)
        ones_w = consts.tile([128, 128], BF16)
        nc.vector.memset(ones_w, 1.0)
        warm_ps = ps_o.tile([128, 512], FP32, tag="po0")
        for _ in range(20):
            nc.tensor.matmul(warm_ps[:, 0:128], ones_w, ones_w[:, 0:128], start=True, stop=True)
        beta2_b = consts.tile([128, C], FP32)
        nc.sync.dma_start(out=beta2_b, in_=beta2_d.to_broadcast((128, C)))
        b2_b = consts.tile([128, C], FP32)
        nc.sync.dma_start(out=b2_b, in_=b2_d.to_broadcast((128, C)))
        b1_sb = consts.tile([128, HT], FP32)
        nc.sync.dma_start(out=b1_sb, in_=b1_d.rearrange("(a p) one -> p (a one)", p=128))
        eps_sb = consts.tile([128, 1], FP32)
        nc.vector.memset(eps_sb, EPS)

        at_sb = []
        for ci in range(CT):
            t = persist.tile([128, ROWS], BF16, tag=f"at{ci}")
            nc.sync.dma_start(out=t, in_=at_d[ci * 128:(ci + 1) * 128, :])
            at_sb.append(t)
        wp_sb = []
        for ci in range(CT):
            t = persist.tile([128, C], BF16, tag=f"wp{ci}")
            nc.sync.dma_start(out=t, in_=wp_d[ci * 128:(ci + 1) * 128, :])
            wp_sb.append(t)
        xr_sb = []
        for m in range(MT):
            t = persist.tile([128, C], FP32, tag=f"xr{m}")
            nc.sync.dma_start(out=t, in_=xr_d[m * 128:(m + 1) * 128, :])
            xr_sb.append(t)

        # ---- proj + residual + bp -> x2 (fp32) ----
        x2_sb = []
        for m in range(MT):
            x2 = persist.tile([128, C], FP32, tag=f"x2{m}")
            for n in range(C // 512):
                nsl = slice(n * 512, (n + 1) * 512)
                pp = ps_p.tile([128, 512], FP32, tag="mm")
                for ci in range(CT):
                    nc.tensor.matmul(pp, at_sb[ci][:, m * 128:(m + 1) * 128],
                                     wp_sb[ci][:, nsl],
                                     start=(ci == 0), stop=(ci == CT - 1))
                nc.vector.tensor_add(x2[:, nsl], pp, xr_sb[m][:, nsl])
            x2_sb.append(x2)

        # ---- LN2 -> h2 (bf16) ----
        h2_sb = []
        for m in range(MT):
            stats_t = small.tile([128, 2, 6], FP32, tag="bnstats")
            for g in range(2):
                nc.vector.bn_stats(out=stats_t[:, g, :], in_=x2_sb[m][:, g * 512:(g + 1) * 512])
            mv = small.tile([128, 2], FP32, tag="mv")
            nc.vector.bn_aggr(out=mv, in_=stats_t)
            lnv = small.tile([128, 1], FP32, tag="lnv")
            nc.scalar.activation(out=lnv, in_=mv[:, 1:2], func=AF.Ln,
                                 bias=eps_sb, scale=1.0)
            rstd = small.tile([128, 1], FP32, tag="rstd2")
            nc.scalar.activation(out=rstd, in_=lnv, func=AF.Exp, scale=-0.5)
            nmr = small.tile([128, 1], FP32, tag="nmr")
            nc.vector.tensor_mul(nmr, mv[:, 0:1], rstd)
            nc.vector.tensor_scalar_mul(nmr, nmr, -1.0)
            h2 = persist.tile([128, C], BF16, tag=f"h2{m}")
            nc.scalar.activation(out=h2, in_=x2_sb[m], func=AF.Identity,
                                 bias=nmr, scale=rstd)
            nc.vector.tensor_add(h2, h2, beta2_b)
            h2_sb.append(h2)

        # ---- transpose h2 -> h2T [128, ROWS] x CT ----
        h2t_sb = []
        for ci in range(CT):
            h2t = persist.tile([128, ROWS], BF16, tag=f"h2t{ci}")
            h2t_sb.append(h2t)
        for m in range(MT):
            for ci in range(CT):
                pt = ps_tr.tile([128, 128], BF16, tag="pt")
                nc.tensor.transpose(pt, h2_sb[m][:, ci * 128:(ci + 1) * 128], ident)
                nc.scalar.copy(out=h2t_sb[ci][:, m * 128:(m + 1) * 128], in_=pt)

        # ---- FFN1: H1T[ht] = relu(W1g.T @ h2T + b1) ----
        h1t_sb = []
        for ht in range(HT):
            w1t = wstream.tile([128, CT, 128], BF16, tag="w1t")
            nc.sync.dma_start(out=w1t,
                              in_=w1_d[:, ht, :].rearrange("p (a n) -> p a n", a=CT))
            ph = ps_p.tile([128, 512], FP32, tag="mm")
            for ci in range(CT):
                nc.tensor.matmul(ph, w1t[:, ci, :], h2t_sb[ci],
                                 start=(ci == 0), stop=(ci == CT - 1))
            h1 = persist.tile([128, ROWS], BF16, tag=f"h1t{ht}")
            nc.scalar.activation(out=h1, in_=ph, func=AF.Relu,
                                 bias=b1_sb[:, ht:ht + 1], scale=1.0)
            h1t_sb.append(h1)

        # ---- FFN2 + residual + b2 -> out ----
        for n in range(C // 512):
            nsl = slice(n * 512, (n + 1) * 512)
            pos = []
            for m in range(MT):
                po = ps_o.tile([128, 512], FP32, tag=f"po{m}")
                pos.append(po)
            for ht in range(HT):
                w2t = wstream.tile([128, 512], BF16, tag="w2t")
                nc.sync.dma_start(out=w2t, in_=w2_d[ht * 128:(ht + 1) * 128, nsl])
                for m in range(MT):
                    nc.tensor.matmul(pos[m], h1t_sb[ht][:, m * 128:(m + 1) * 128], w2t,
                                     start=(ht == 0), stop=(ht == HT - 1))
            for m in range(MT):
                ot = tmp.tile([128, 512], FP32, tag="ot")
                nc.vector.tensor_add(ot, pos[m], x2_sb[m][:, nsl])
                nc.vector.tensor_add(ot, ot, b2_b[:, nsl])
                nc.sync.dma_start(out=out_d[m * 128:(m + 1) * 128, nsl], in_=ot)
    nc.compile()
    return nc


# ---------------- host glue ----------------

def prep_l1_inputs(inputs):
    x = np.asarray(inputs["x"], np.float32).reshape(TOK, C)
    g1 = np.asarray(inputs["g1"], np.float32)
    beta1 = np.asarray(inputs["beta1"], np.float32)
    xt = np.ascontiguousarray(x.T).astype(bf16)
    wq = (g1[:, None] * np.asarray(inputs["Wq"], np.float32)).astype(bf16)
    wk = (g1[:, None] * np.asarray(inputs["Wk"], np.float32)).astype(bf16)
    wv = (g1[:, None] * np.asarray(inputs["Wv"], np.float32)).astype(bf16)
    tri = np.triu(np.ones((128, 128), np.float32)).astype(bf16)
    in_maps = []
    for c in range(NCORES):
        csl = slice(c * 128, (c + 1) * 128)
        nws = np.stack([-wq[:, csl].astype(np.float32).sum(0),
                        -wk[:, csl].astype(np.float32).sum(0),
                        -wv[:, csl].astype(np.float32).sum(0)], axis=1)
        wb = np.stack([wq[:, csl].astype(np.float32).T @ beta1,
                       wk[:, csl].astype(np.float32).T @ beta1,
                       wv[:, csl].astype(np.float32).T @ beta1], axis=1)
        in_maps.append({
            "xt": xt,
            "wq": np.ascontiguousarray(wq[:, csl]),
            "wk": np.ascontiguousarray(wk[:, csl]),
            "wv": np.ascontiguousarray(wv[:, csl]),
            "nws": np.ascontiguousarray(nws.astype(np.float32)),
            "wb": np.ascontiguousarray(wb.astype(np.float32)),
            "tri": tri,
        })
    return in_maps


def prep_l2_inputs(inputs, attn_t):
    attn_t = np.ascontiguousarray(np.asarray(attn_t, bf16))
    x = np.asarray(inputs["x"], np.float32).reshape(TOK, C)
    g2 = np.asarray(inputs["g2"], np.float32)
    wp = np.asarray(inputs["Wp"], np.float32).astype(bf16)
    w1 = (g2[:, None] * np.asarray(inputs["W1"], np.float32)).astype(bf16)
    w1q = np.ascontiguousarray(
        w1.reshape(CT, 128, HT, 128).transpose(1, 2, 0, 3).reshape(128, HT, CT * 128))
    w2 = np.asarray(inputs["W2"], np.float32).astype(bf16)
    b1 = np.ascontiguousarray(np.asarray(inputs["b1"], np.float32).reshape(HID, 1))
    x = x + np.asarray(inputs["bp"], np.float32)[None, :]
    beta2 = np.ascontiguousarray(np.asarray(inputs["beta2"], np.float32).reshape(1, C))
    b2 = np.ascontiguousarray(np.asarray(inputs["b2"], np.float32).reshape(1, C))
    in_maps = []
    for c in range(NCORES):
        rsl = slice(c * ROWS, (c + 1) * ROWS)
        in_maps.append({
            "at": np.ascontiguousarray(attn_t[:, rsl]),
            "wp": wp,
            "xr": np.ascontiguousarray(x[rsl, :]),
            "w1q": w1q,
            "w2": w2,
            "b1": b1,
            "beta2": beta2,
            "b2": b2,
        })
    return in_maps


_CACHE = {}


def _get_programs(use_beta):
    key = ("progs", bool(use_beta))
    if key not in _CACHE:
        nc1 = build_l1(use_beta=use_beta)
        nc2 = build_l2()
        _CACHE[key] = (nc1, nc2)
    return _CACHE[key]


def kernel(**inputs):
    from concourse.bass_utils import run_bass_kernel_spmd

    inputs = {k: np.asarray(v) for k, v in inputs.items()}
    use_beta = bool(np.any(np.asarray(inputs["beta1"], np.float32) != 0.0))
    nc1, nc2 = _get_programs(use_beta)
    core_ids = list(range(NCORES))

    r1 = run_bass_kernel_spmd(nc1, prep_l1_inputs(inputs), core_ids)
    attn_t = np.concatenate(
        [np.asarray(r1.results[c]["attn_out"]) for c in range(NCORES)], axis=0)

    r2 = run_bass_kernel_spmd(nc2, prep_l2_inputs(inputs, attn_t), core_ids)
    out = np.concatenate(
        [np.asarray(r2.results[c]["out_rows"]) for c in range(NCORES)], axis=0)
    return np.ascontiguousarray(out.reshape(B, T, C).astype(np.float32))

